# revision 1
# baseline (speedup 1.0000x reference)
"""GQA attention block (B=2, L=2048, D=4096, H=32, HKV=8, RoPE, causal) on 8
Trainium2 NeuronCores.

Sharding: core c -> batch b=c//4, head-group g=c%4 (8 Q heads + 2 KV heads per
core).  Each core computes x[b] @ wq_g/wk_g/wv_g projections, RoPE, causal
attention for its heads, and a partial output projection against its slice of
wo (row-sharded contraction).  The host sums the 4 partials per batch element
(the all-reduce of the tensor-parallel output projection, done at unshard).

Device layouts put every matmul contraction on the partition axis; the host
pre-tiles x and all weights into the exact SBUF tile layouts so every DMA is
a single fully-contiguous read.  wq/wk rows are pair-permuted ([evens|odds]
per head) so RoPE becomes a partition half-swap, folded into partition-offset
operands of the sin multiply (no explicit swap copies).

Scores are computed transposed, S^T[j, l] = K^T.T @ Q^T, so softmax probs
feed the PV matmul with no on-chip transposes.  The softmax denominator
accumulates through an all-ones stationary matmul over the same E^T tiles
(partition-broadcast for free); normalization is reciprocal_approx_fast +
multiply.  Causality: fully-masked key tiles are skipped; diagonal tiles are
zeroed post-exp with a gpsimd affine_select (exp(s+m) == exp(s)*[m==0]
exactly for the 0/-1e9 mask).  Score matmuls are issued LOOKAHEAD tiles
ahead of the PV/denominator matmuls so the scalar-engine exp latency stays
off the PE critical path.

All matmuls run in float32r (full-rate fp32 mode at moving-dim 512,
~1.6e-4 rel err measured on hardware).
"""

import numpy as np

import concourse.mybir as mybir
import concourse.tile as tile
from concourse import bacc, bass_utils

B, L, D = 2, 2048, 4096
H, HKV, HD = 32, 8, 128
NCORES = 8
GROUPS = 4                # head groups (cores per batch element)
QH = H // GROUPS          # 8 q heads per core
KVH = HKV // GROUPS       # 2 kv heads per core
NM = QH + 2 * KVH         # 12 projection m-tiles per core (q0..7, k0..1, v0..1)
LC = 512                  # l-chunk (matmul moving free dim)
DT = D // 128             # 32 contraction tiles for projections
SCALE = 1.0 / float(np.sqrt(HD))
LOOKAHEAD = 3             # score-matmul tiles in flight ahead of PV

f32 = mybir.dt.float32
f32r = mybir.dt.float32r


def build_nc(seq_len=L):
    nlc = seq_len // LC
    njt_all = seq_len // 128

    lc_pairs0 = [
        [lc for lc in (2 * i, 2 * i + 1) if lc < nlc] for i in range((nlc + 1) // 2)
    ]
    max_plc = max(len(p) for p in lc_pairs0)
    nc = bacc.Bacc(trn_type="TRN2")
    # host-pre-tiled operands: every DMA below is a contiguous read
    x_tl = nc.dram_tensor(
        "x_tl", [len(lc_pairs0) * 8, 128, 4 * max_plc * LC], f32, kind="ExternalInput"
    )
    wqkv_tl = nc.dram_tensor(
        "wqkv_tl", [NM * 2, 128, 16 * 128], f32, kind="ExternalInput"
    )
    wo_tl = nc.dram_tensor("wo_tl", [D // 128, 128, QH * 128], f32, kind="ExternalInput")
    cosT = nc.dram_tensor("cosT", [64, seq_len], f32, kind="ExternalInput")
    sinT = nc.dram_tensor("sinT", [64, seq_len], f32, kind="ExternalInput")
    ones128 = nc.dram_tensor("ones128", [128, 128], f32, kind="ExternalInput")
    ident = nc.dram_tensor("ident", [128, 128], f32, kind="ExternalInput")
    outT = nc.dram_tensor("outT", [D, seq_len], f32, kind="ExternalOutput")

    with tile.TileContext(nc) as tc:
        with (
            tc.tile_pool(name="persist", bufs=1) as pp,
            tc.tile_pool(name="xp", bufs=1) as xp,
            tc.tile_pool(name="qp", bufs=1) as qp,
            tc.tile_pool(name="op", bufs=1) as op_,
            tc.tile_pool(name="wp", bufs=2) as wp,
            tc.tile_pool(name="ep", bufs=2) as ep,
            tc.tile_pool(name="tp", bufs=1) as tp,
            tc.tile_pool(name="outp", bufs=1) as outp,
            tc.tile_pool(name="mmps", bufs=4, space="PSUM") as mmps,
            tc.tile_pool(name="ops", bufs=2, space="PSUM") as ops_,
            tc.tile_pool(name="dps", bufs=2, space="PSUM") as dps,
        ):
            kT_t = {
                (kv, lc): pp.tile(
                    [128, LC], f32r, tag=f"kT_{kv}_{lc}", name=f"kT_{kv}_{lc}"
                )
                for kv in range(KVH) for lc in range(nlc)
            }
            v_t = {
                lc: pp.tile(
                    [128, 4, KVH * HD], f32r, tag=f"v_{lc}", name=f"v_{lc}"
                )
                for lc in range(nlc)
            }
            cs2 = pp.tile([128, seq_len], f32)
            sn2 = pp.tile([128, seq_len], f32)
            o128 = pp.tile([128, 128], f32r)
            idt = pp.tile([128, 128], f32r)

            nc.scalar.dma_start(cs2[0:64, :], cosT.ap())
            nc.scalar.dma_start(cs2[64:128, :], cosT.ap())
            nc.scalar.dma_start(sn2[0:64, :], sinT.ap())
            nc.scalar.dma_start(sn2[64:128, :], sinT.ap())
            # rotate-half form: out = q*cs2 + swap(q)*sn2 with sn2 = [-sin | sin]
            nc.vector.tensor_scalar_mul(sn2[0:64, :], sn2[0:64, :], -1.0)
            nc.scalar.dma_start(o128[:], ones128.ap().bitcast(f32r))
            nc.scalar.dma_start(idt[:], ident.ap().bitcast(f32r))

            lc_pairs = [
                [lc for lc in (2 * i, 2 * i + 1) if lc < nlc]
                for i in range((nlc + 1) // 2)
            ]

            for pi, lcs in enumerate(lc_pairs):
                plc = len(lcs)
                o2 = op_.tile([128, QH, plc * LC], f32r, tag="o2")
                q_pr = qp.tile([128, QH, plc * LC], f32r, tag="q")
                # ---- phase 1: projections for the pair, contraction dt-halves ----
                for half in range(2):
                    x_h = xp.tile([128, DT // 2, plc * LC], f32r, tag="x")
                    for quar in range(4):
                        nc.sync.dma_start(
                            x_h[:, quar * 4:(quar + 1) * 4, :],
                            x_tl.ap()[pi * 8 + half * 4 + quar]
                            .rearrange("p (a b) -> p a b", a=4)[:, :, : plc * LC]
                            .bitcast(f32r),
                        )
                    for mi in range(NM):
                        kind = "q" if mi < QH else ("k" if mi < QH + KVH else "v")
                        m = mi if mi < QH else (mi - QH if kind == "k" else mi - QH - KVH)
                        wt = wp.tile([128, 16 * 128], f32r, tag="w")
                        nc.sync.dma_start(
                            wt[:], wqkv_tl.ap()[mi * 2 + half].bitcast(f32r)
                        )
                        for lci, lc in enumerate(lcs):
                            ps = mmps.tile([128, LC], f32, tag="mm")
                            for dt8 in range(16):
                                nc.tensor.matmul(
                                    ps[:],
                                    wt[:, dt8 * 128:(dt8 + 1) * 128],
                                    x_h[:, dt8, lci * LC:(lci + 1) * LC],
                                    start=(dt8 == 0), stop=(dt8 == 15),
                                )
                            if kind in ("q", "k"):
                                lsl = slice(lc * LC, (lc + 1) * LC)
                                t1 = tp.tile([128, LC], f32, tag="t1")
                                nc.vector.tensor_mul(t1[:], ps[:], cs2[:, lsl])
                                t2 = tp.tile([128, LC], f32, tag="t2")
                                nc.vector.tensor_mul(
                                    t2[0:64, :], ps[64:128, :], sn2[0:64, lsl]
                                )
                                nc.vector.tensor_mul(
                                    t2[64:128, :], ps[0:64, :], sn2[64:128, lsl]
                                )
                                dst = (
                                    q_pr[:, m, lci * LC:(lci + 1) * LC]
                                    if kind == "q"
                                    else kT_t[(m, lc)][:]
                                )
                                if half == 0:
                                    nc.vector.tensor_tensor(
                                        dst, t1[:], t2[:], mybir.AluOpType.add
                                    )
                                else:
                                    nc.vector.tensor_tensor(
                                        dst, dst, t1[:], mybir.AluOpType.add
                                    )
                                    nc.vector.tensor_tensor(
                                        dst, dst, t2[:], mybir.AluOpType.add
                                    )
                            else:
                                vt = tp.tile([128, LC], f32r, tag="vt")
                                nc.vector.tensor_copy(vt[:], ps[:])
                                for jj in range(4):
                                    pt = mmps.tile([128, 128], f32r, tag="mm")
                                    nc.tensor.transpose(
                                        pt[:], vt[:, jj * 128:(jj + 1) * 128], idt[:]
                                    )
                                    dstv = v_t[lc][:, jj, m * 128:(m + 1) * 128]
                                    if half == 0:
                                        nc.vector.tensor_copy(dstv, pt[:])
                                    else:
                                        nc.vector.tensor_tensor(
                                            dstv, dstv, pt[:], mybir.AluOpType.add
                                        )
                for lci, lc in enumerate(lcs):
                    # ---- phase 2: causal attention for queries in this l-chunk ----
                    njt = 4 * (lc + 1)
                    for h in range(QH):
                        kv = h // (QH // KVH)
                        po = ops_.tile([128, LC], f32, tag="po")
                        pden = dps.tile([128, LC], f32, tag="pden")
                        e_tiles = {}

                        def emit_score(jt, h=h, kv=kv, e_tiles=e_tiles, lc=lc):
                            psS = mmps.tile([128, LC], f32, tag="mm")
                            nc.tensor.matmul(
                                psS[:],
                                kT_t[(kv, jt // 4)][:, (jt % 4) * 128:(jt % 4 + 1) * 128],
                                q_pr[:, h, lci * LC:(lci + 1) * LC],
                                start=True, stop=True,
                            )
                            e = ep.tile([128, LC], f32r, tag="e")
                            nc.scalar.activation(
                                e[:], psS[:], mybir.ActivationFunctionType.Exp,
                                scale=SCALE,
                            )
                            dg = jt - 4 * lc
                            if dg >= 0:
                                # causal: zero E where key j > query l
                                # value(p, y) = -p + y - 128*dg ; keep when >= 0
                                nc.gpsimd.affine_select(
                                    out=e[:], in_=e[:],
                                    compare_op=mybir.AluOpType.is_ge,
                                    fill=0.0,
                                    base=-128 * dg,
                                    pattern=[[1, LC]],
                                    channel_multiplier=-1,
                                )
                            e_tiles[jt] = e

                        for jt in range(min(LOOKAHEAD, njt)):
                            emit_score(jt)
                        for jt in range(njt):
                            if jt + LOOKAHEAD < njt:
                                emit_score(jt + LOOKAHEAD)
                            e = e_tiles.pop(jt)
                            nc.tensor.matmul(
                                po[:],
                                v_t[jt // 4][:, jt % 4, kv * 128:(kv + 1) * 128],
                                e[:],
                                start=(jt == 0), stop=(jt == njt - 1),
                            )
                            nc.tensor.matmul(
                                pden[:], o128[:], e[:],
                                start=(jt == 0), stop=(jt == njt - 1),
                            )
                        rec = tp.tile([128, LC], f32, tag="rec")
                        nc.vector.reciprocal_approx_fast(out=rec[:], in_=pden[:])
                        nc.vector.tensor_mul(
                            o2[:, h, lci * LC:(lci + 1) * LC], po[:], rec[:]
                        )
                # ---- phase 3: partial output projection for the pair ----
                for nt in range(D // 128):
                    wo_t = wp.tile([128, QH * 128], f32r, tag="w")
                    nc.scalar.dma_start(wo_t[:], wo_tl.ap()[nt].bitcast(f32r))
                    for lci, lc in enumerate(lcs):
                        pso = mmps.tile([128, LC], f32, tag="mm")
                        for h in range(QH):
                            nc.tensor.matmul(
                                pso[:], wo_t[:, h * 128:(h + 1) * 128],
                                o2[:, h, lci * LC:(lci + 1) * LC],
                                start=(h == 0), stop=(h == QH - 1),
                            )
                        ob = outp.tile([128, LC], f32, tag="ob")
                        nc.vector.tensor_copy(ob[:], pso[:])
                        nc.sync.dma_start(
                            outT.ap()[nt * 128:(nt + 1) * 128, lc * LC:(lc + 1) * LC],
                            ob[:],
                        )
    nc.compile()
    return nc


_PERM = np.concatenate([np.arange(0, HD, 2), np.arange(1, HD, 2)])


def _tile_weight(wT):
    """[D, M] (transposed weight) -> [M//128 * 2, 128, 16*128] contiguous tiles:
    tile (m, half)[p, dt8, mc] = wT[(half*16+dt8)*128 + p, m*128 + mc]."""
    Dd, M = wT.shape
    w = wT.reshape(2, 16, 128, M // 128, 128)         # [half, dt8, p, m, mc]
    w = w.transpose(3, 0, 2, 1, 4)                     # [m, half, p, dt8, mc]
    return np.ascontiguousarray(w.reshape(M // 128 * 2, 128, 16 * 128), np.float32)


def shard_inputs(x, wq, wk, wv, wo, cos, sin, mask, seq_len=L):
    """Build the 8 per-core input maps (host pre-tiling)."""
    nlc = seq_len // LC
    cosT = np.ascontiguousarray(cos[:seq_len].T, dtype=np.float32)
    sinT = np.ascontiguousarray(sin[:seq_len].T, dtype=np.float32)
    ones128 = np.ones((128, 128), np.float32)
    ident = np.eye(128, dtype=np.float32)

    lc_pairs = [
        [lc for lc in (2 * i, 2 * i + 1) if lc < nlc] for i in range((nlc + 1) // 2)
    ]
    max_plc = max(len(p) for p in lc_pairs)
    x_tls = []
    for b in range(B):
        xT = x[b, :seq_len].T.astype(np.float32)       # [D, seq]
        xv = xT.reshape(8, 4, 128, seq_len)            # [hq(half*4+quar), dt4, p, l]
        x_tl = np.zeros((len(lc_pairs) * 8, 128, 4 * max_plc * LC), np.float32)
        for pi, lcs in enumerate(lc_pairs):
            cols = np.concatenate([np.arange(lc * LC, (lc + 1) * LC) for lc in lcs])
            blk = xv[:, :, :, cols]                    # [hq, dt4, p, plc*LC]
            blk = blk.transpose(0, 2, 1, 3)            # [hq, p, dt4, plc*LC]
            x_tl[pi * 8:(pi + 1) * 8, :, : len(cols) * 4] = blk.reshape(8, 128, -1)
        x_tls.append(x_tl)

    def permute_rows(w):
        nh = w.shape[0] // HD
        wp_ = w.reshape(nh, HD, -1)[:, _PERM, :]
        return wp_.reshape(w.shape)

    in_maps = []
    for c in range(NCORES):
        b, g = divmod(c, GROUPS)
        wq_g = permute_rows(wq[QH * HD * g:QH * HD * (g + 1)])
        wk_g = permute_rows(wk[KVH * HD * g:KVH * HD * (g + 1)])
        wv_g = wv[KVH * HD * g:KVH * HD * (g + 1)]
        wo_g = wo[:, QH * HD * g:QH * HD * (g + 1)]
        wqkv_tl = np.concatenate(
            [_tile_weight(wq_g.T), _tile_weight(wk_g.T), _tile_weight(wv_g.T)], axis=0
        )
        woT = wo_g.T.astype(np.float32)                # [1024, D]
        wov = woT.reshape(QH, 128, D // 128, 128)      # [kt, p, nt, n]
        wov = wov.transpose(2, 1, 0, 3)                # [nt, p, kt, n]
        wo_tl = np.ascontiguousarray(wov.reshape(D // 128, 128, QH * 128), np.float32)
        in_maps.append({
            "x_tl": x_tls[b],
            "wqkv_tl": wqkv_tl,
            "wo_tl": wo_tl,
            "cosT": cosT,
            "sinT": sinT,
            "ones128": ones128,
            "ident": ident,
        })
    return in_maps


def gather_output(results, seq_len=L):
    out = np.zeros((B, seq_len, D), np.float32)
    for c in range(NCORES):
        b = c // GROUPS
        out[b] += results[c]["outT"].T
    return out


_nc_cache = {}


def _get_nc(seq_len=L):
    if seq_len not in _nc_cache:
        _nc_cache[seq_len] = build_nc(seq_len)
    return _nc_cache[seq_len]


def run_sharded(inputs, trace=False, tmpdir=None):
    nc = _get_nc()
    in_maps = shard_inputs(**inputs)
    res = bass_utils.run_bass_kernel_spmd(
        nc, in_maps, core_ids=list(range(NCORES)), trace=trace, tmpdir=tmpdir
    )
    return gather_output(res.results), res


def kernel(**inputs) -> np.ndarray:
    out, _ = run_sharded(inputs)
    return out

